# revision 7
# baseline (speedup 1.0000x reference)
"""Trainium2 Bass kernel for nn_BoundaryDetectionLoss.

Computes, for start/end (probs, targets) pairs of shape (64, 131072):
    w   = 1 + exp(-dist_to_nearest_boundary / 5)     (distance transform)
    bce = (1-z)*x + (1+z)*softplus(-x)               (pos_weight = 2)
    loss = mean(bce * w)   per pair; total = (start_loss + end_loss)/2

Identity used on device (g = softplus(+x), e = exp(-dist/5), z*e == z):
    bce*w = g*(1 + e + 2z) - 4*z*x
so with host-staged z2 = 2*z (zero-padded halo) and e2 = 2e from the
decayed-max scans:
    sum(bce*w) = sum(g) + 0.5*sum(g*e2) + sum(z2*g) - 2*sum(z2*x)

Device algorithm (per core, data-parallel over 8 rows of B=64):
  - e2[t] = 2*exp(-dist[t]/5) as a decayed-max field with two DVE
    tensor_tensor_scan passes (op0=mult by a=exp(-1/5), op1=max), 128-element
    halo per tile (contributions beyond ~84 positions underflow below fp16).
    Scans are DVE-only: TensorTensorScanArith is not a legal GPSIMD opcode.
  - g = softplus(x) = ln(1+exp(x)) on ACT (Exp+Ln share one LUT set; walrus
    has no softplus set); the Ln's accum_out gives sum(g) per partition free.
  - Dots sum(z2*g), sum(z2*x), sum(e2*g) on the PE via 128-wide block
    matmuls accumulating lhsT^T @ rhs into PSUM; host sums the diagonals.
  - Inputs staged fp16 by the host (pure dtype conversion + padding):
    halves HBM traffic vs f32 and feeds the PE directly.
"""

import sys

for _p in ("/opt/trn_rl_repo", "/root/.axon_site/_ro/trn_rl_repo"):
    if _p not in sys.path:
        sys.path.append(_p)

import numpy as np

# ---------------------------------------------------------------- config
B_FULL = 64
T_FULL = 131072
N_CORES = 8
ROWS = B_FULL // N_CORES  # 8 rows per core
DECAY = float(np.float16(np.exp(np.float32(-0.2))))  # a = exp(-1/5) in fp16
# two fp16 DECAY values bit-packed as one f32 (memset the const tile at
# half the DVE cycles by writing f32 pairs)
DECAY_PAIR = float(
    np.frombuffer(np.array([DECAY, DECAY], np.float16).tobytes(), np.float32)[0]
)


class Cfg:
    def __init__(self, rows=8, chunks=16, j_tiles=2, tile_len=4096, halo=64,
                 z_dt="float8e4", x_dt="float8e4", e2_dt="float8e4"):
        self.rows = rows
        self.chunks = chunks
        self.j_tiles = j_tiles
        self.tile_len = tile_len
        self.halo = halo
        self.chunk_len = j_tiles * tile_len
        self.T = chunks * self.chunk_len
        self.parts = rows * chunks
        assert self.parts <= 128
        self.blk = 128
        self.n_blk = tile_len // self.blk
        assert halo <= tile_len
        self.z_dt = z_dt
        self.x_dt = x_dt
        self.e2_dt = e2_dt
        self.n_tiles = 2 * j_tiles


PROD_CFG = Cfg()
PAIRS = (("start_probs", "start_targets"), ("end_probs", "end_targets"))


def _build_body(nc, tc, cfg, dram_in, acc, psums_z, psums_e,
                const_v, pools, bass, mybir):
    f16 = mybir.dt.float16
    AF = mybir.ActivationFunctionType
    OP = mybir.AluOpType
    zpool, gpool, epool, e2pool, tpool = pools
    P, TL, H = cfg.parts, cfg.tile_len, cfg.halo
    W = TL + 2 * H
    zdt = getattr(mybir.dt, cfg.z_dt)
    xdt = getattr(mybir.dt, cfg.x_dt)
    Tp = cfg.T + 2 * H  # padded row length

    # ---- phase 1: DMA loads + ACT softplus, tile-major (zw first: the DVE
    # scan chain is the critical path and consumes zw earliest)
    tiles = []
    for pi, (px, pz) in enumerate(PAIRS):
        xd, zd = dram_in[px], dram_in[pz]
        x4 = xd[:].rearrange(
            "r (c j f) -> (r c) j f", c=cfg.chunks, j=cfg.j_tiles
        )
        for j in range(cfg.j_tiles):
            ti = pi * cfg.j_tiles + j
            # window for partition (r, c): padded cols
            # [c*chunk_len + j*TL, +W) — always in-bounds by padding
            zw = zpool.tile([P, W], zdt, tag="zw")
            zwin = bass.AP(
                zd,
                j * TL,
                [[Tp, cfg.rows], [cfg.chunk_len, cfg.chunks], [1, W]],
            )
            nc.sync.dma_start(zw[:], zwin)

            # gx = [g | x]: DMA x into the right half, ACT writes
            # g = softplus(x) = ln(1 + exp(x)) into the left half, so one
            # 256-wide PE moving operand covers both z2@g and z2@x.
            gx = gpool.tile([P, 2 * TL], xdt, tag="gx")
            nc.sync.dma_start(gx[:, TL : 2 * TL], x4[:, j, :])
            texp = tpool.tile([P, TL], f16, tag="texp")
            nc.scalar.activation(texp[:], gx[:, TL : 2 * TL], AF.Exp)
            nc.scalar.activation(
                gx[:, 0:TL], texp[:], AF.Ln, bias=1.0,
                accum_out=acc[:, ti : ti + 1],
            )
            tiles.append((pi, j, zw, gx))

    # ---- phase 2: DVE scans (fwd full window, rev only [H, W) reversed).
    # STT-class ops have one ISA sync-wait slot; a 1-element same-engine
    # tensor_tensor touching the same tiles absorbs the waits so program
    # order covers the scan.
    e2s = []
    e2dt = getattr(mybir.dt, cfg.e2_dt)
    cb_fwd = const_v[:].broadcast_to((P, W))
    cb_rev = const_v[:].broadcast_to((P, W - H))  # broadcast: no reversal needed
    for pi, j, zw, gx in tiles:
        ef = epool.tile([P, W], f16, tag="ef")
        nc.vector.tensor_tensor(ef[:, 0:1], zw[:, 0:1], const_v[:, 0:1], OP.mult)
        nc.vector.tensor_tensor_scan(ef[:], cb_fwd, zw[:], 0.0, OP.mult, OP.max)
        e2 = e2pool.tile([P, W], e2dt, tag="e2")
        nc.vector.tensor_tensor(e2[:, H : H + 1], ef[:, H : H + 1],
                                const_v[:, 0:1], OP.mult)
        nc.vector.tensor_tensor_scan(
            e2[:, W - 1 : H - 1 : -1], cb_rev,
            ef[:, W - 1 : H - 1 : -1], 0.0, OP.mult, OP.max
        )
        e2s.append(e2)

    # ---- phase 3: PE matmuls. z-mms of a tile depend only on (zw, gx);
    # e-mms additionally on that tile's rev scan. Order z(0), z(1), e(0),
    # z(2), e(1), z(3), e(2), e(3) keeps the PE fed while scans complete.
    def z_mms(ti):
        pi, j, zw, gx = tiles[ti]
        gx3 = gx[:].rearrange("p (g f) -> p g f", g=2)
        for b in range(cfg.n_blk):
            s = slice(b * cfg.blk, (b + 1) * cfg.blk)
            hs = slice(H + b * cfg.blk, H + (b + 1) * cfg.blk)
            first = j == 0 and b == 0
            last = j == cfg.j_tiles - 1 and b == cfg.n_blk - 1
            nc.tensor.matmul(
                psums_z[pi][:], zw[:, hs], gx3[:, :, s], start=first, stop=last
            )

    def e_mms(ti):
        pi, j, zw, gx = tiles[ti]
        e2 = e2s[ti]
        for b in range(cfg.n_blk):
            s = slice(b * cfg.blk, (b + 1) * cfg.blk)
            hs = slice(H + b * cfg.blk, H + (b + 1) * cfg.blk)
            first = j == 0 and b == 0
            last = j == cfg.j_tiles - 1 and b == cfg.n_blk - 1
            nc.tensor.matmul(
                psums_e[pi][:], e2[:, hs], gx[:, s], start=first, stop=last
            )

    nt = cfg.n_tiles
    order = []
    for ti in range(nt):
        order.append(("z", ti))
        if ti >= 2:
            order.append(("e", ti - 2))
    order += [("e", nt - 2), ("e", nt - 1)]
    for kind, ti in order:
        (z_mms if kind == "z" else e_mms)(ti)


def build_nc(cfg: Cfg, split_waits=True, loop_n=1):
    """Build the per-core Bass program. Returns nc."""
    import concourse.bass as bass
    import concourse.tile as tile
    import concourse.mybir as mybir

    f32 = mybir.dt.float32
    f16 = mybir.dt.float16

    P, TL, H = cfg.parts, cfg.tile_len, cfg.halo
    W = TL + 2 * H  # scan window length
    zdt = getattr(mybir.dt, cfg.z_dt)
    xdt = getattr(mybir.dt, cfg.x_dt)

    nc = bass.Bass()
    dram_in = {}
    for px, pz in PAIRS:
        dram_in[px] = nc.dram_tensor(px, [cfg.rows, cfg.T], xdt, kind="ExternalInput")
        # targets arrive host-staged as 2*z, padded with H zeros on each
        # side of every row
        dram_in[pz] = nc.dram_tensor(
            pz, [cfg.rows, cfg.T + 2 * cfg.halo], zdt, kind="ExternalInput"
        )
    n_acc = cfg.n_tiles  # one sum(g) accum column per (pair, j)
    acc_out = nc.dram_tensor("acc", [P, n_acc], f32, kind="ExternalOutput")
    dots_z_out = nc.dram_tensor(
        "dots_z", [2, cfg.blk, 2 * cfg.blk], f32, kind="ExternalOutput"
    )
    dots_e_out = nc.dram_tensor(
        "dots_e", [2, cfg.blk, cfg.blk], f32, kind="ExternalOutput"
    )

    with tile.TileContext(nc) as tc:
        with (
            tc.tile_pool(name="const", bufs=1) as cpool,
            tc.tile_pool(name="zwin", bufs=4) as zpool,
            tc.tile_pool(name="gxp", bufs=4) as gpool,
            tc.tile_pool(name="efp", bufs=2) as epool,
            tc.tile_pool(name="e2p", bufs=3) as e2pool,
            tc.tile_pool(name="texp", bufs=2) as tpool,
            tc.tile_pool(name="accp", bufs=1) as apool,
            tc.tile_pool(name="psum", bufs=1, space="PSUM") as ppool,
            tc.tile_pool(name="outp", bufs=1) as opool,
        ):
            # decay constant: a single element per partition, read by the
            # scans through a stride-0 broadcast AP (no wide memset needed)
            const_v = cpool.tile([P, 1], f16, tag="cav")
            nc.vector.memset(const_v[:], DECAY)

            acc = apool.tile([P, n_acc], f32, tag="acc")

            psums_z = [
                ppool.tile([cfg.blk, 2 * cfg.blk], f32, tag=f"pz{i}", name=f"pz{i}")
                for i in range(2)
            ]
            psums_e = [
                ppool.tile([cfg.blk, cfg.blk], f32, tag=f"pe{i}", name=f"pe{i}")
                for i in range(2)
            ]

            import contextlib

            loop_cm = (
                tc.For_i(0, loop_n, 1, hint_engines=(mybir.EngineType.PE,))
                if loop_n > 1
                else contextlib.nullcontext()
            )
            with loop_cm:
                _build_body(nc, tc, cfg, dram_in, acc, psums_z, psums_e,
                            const_v, (zpool, gpool, epool, e2pool, tpool),
                            bass, mybir)

            # --- drain results on ACT (DVE is the critical path and
            # GPSIMD cannot access PSUM; ACT Copy reads PSUM fine)
            AF = mybir.ActivationFunctionType
            nc.sync.dma_start(acc_out[:], acc[:])
            for pi in range(2):
                dz = opool.tile([cfg.blk, 2 * cfg.blk], f32, tag=f"dz{pi}",
                                name=f"dz{pi}")
                nc.scalar.activation(dz[:], psums_z[pi][:], AF.Copy)
                nc.sync.dma_start(dots_z_out[pi, :, :], dz[:])
                de = opool.tile([cfg.blk, cfg.blk], f32, tag=f"de{pi}",
                                name=f"de{pi}")
                nc.scalar.activation(de[:], psums_e[pi][:], AF.Copy)
                nc.sync.dma_start(dots_e_out[pi, :, :], de[:])

    if split_waits:
        _split_multiwaits(nc)
    return nc


def _split_multiwaits(nc):
    """Engine instructions hold at most ONE sync wait in core_v3 ISA structs
    (walrus: 'Too many sync wait commands'). Tile sometimes attaches 2+.
    Move extras onto same-engine NoOps inserted just before the instruction
    (sequencer executes them in order, so semantics are identical)."""
    import concourse.mybir as mybir

    for f in nc.m.functions:
        for blk in f.blocks:
            out = []
            changed = False
            for ins in blk.instructions:
                si = ins.sync_info
                cap = 2 if isinstance(ins, mybir.InstEventSemaphore) else 1
                if si is not None and si.on_wait and len(si.on_wait) > cap:
                    waits = list(si.on_wait)
                    for w in waits[:-cap]:
                        out.append(
                            mybir.InstNoOp(
                                name=nc.get_next_instruction_name(),
                                engine=ins.engine,
                                ins=[],
                                outs=[],
                                sync_info=mybir.SyncInfo(on_wait=[w], on_update=[]),
                            )
                        )
                    ins.sync_info = mybir.SyncInfo(
                        on_wait=waits[-cap:], on_update=list(si.on_update or [])
                    )
                    changed = True
                out.append(ins)
            if changed:
                blk.instructions = out


def host_combine(results, cfg: Cfg):
    """Combine per-core acc/dots into (start_loss, end_loss, total)."""
    n_elem = np.float64(B_FULL) * cfg.T
    losses = []
    B = cfg.blk
    for pi in range(2):
        s = np.float64(0.0)
        for res in results:
            acc = np.asarray(res["acc"], dtype=np.float64)
            dz = np.asarray(res["dots_z"], dtype=np.float64)[pi]
            de = np.asarray(res["dots_e"], dtype=np.float64)[pi]
            cols = [pi * cfg.j_tiles + j for j in range(cfg.j_tiles)]
            s += acc[:, cols].sum()                      # sum(g)
            s += 0.5 * np.trace(de)                      # 0.5*sum(g*e2)
            s += np.trace(dz[:, 0:B])                    # sum(z2*g)
            s -= 2.0 * np.trace(dz[:, B : 2 * B])        # -2*sum(z2*x)
        losses.append(s / n_elem)
    start_loss, end_loss = losses
    total = (start_loss + end_loss) / 2.0
    return (
        np.float32(start_loss),
        np.float32(end_loss),
        np.float32(total),
    )


_NC_CACHE = {}
TRACE = False  # set True (e.g. from test.py) to capture an NTFF profile
LAST_RESULT = None  # BassKernelResults of the most recent run (for profiling)


def _np_dt(name):
    import ml_dtypes

    return {"float16": np.float16, "float8e4": ml_dtypes.float8_e4m3}[name]


def make_in_maps(cfg, inputs):
    """Host staging: shard rows, cast to the device dtypes, pad targets."""
    H = cfg.halo
    xnp, znp = _np_dt(cfg.x_dt), _np_dt(cfg.z_dt)
    in_maps = []
    for k in range(N_CORES):
        rs = slice(k * ROWS, (k + 1) * ROWS)
        m = {}
        for px, pz in PAIRS:
            m[px] = np.ascontiguousarray(np.asarray(inputs[px])[rs]).astype(xnp)
            z2p = np.zeros((ROWS, cfg.T + 2 * H), dtype=znp)
            # targets are exactly 0.0/1.0; 2*z is exact in fp16/fp8
            z2p[:, H : H + cfg.T] = (np.asarray(inputs[pz])[rs] * 2.0).astype(znp)
            m[pz] = z2p
        in_maps.append(m)
    return in_maps


def kernel(**inputs):
    from concourse.bass_utils import run_bass_kernel_spmd

    cfg = PROD_CFG
    key = "prod"
    if key not in _NC_CACHE:
        _NC_CACHE[key] = build_nc(cfg)
    nc = _NC_CACHE[key]

    in_maps = make_in_maps(cfg, inputs)
    res = run_bass_kernel_spmd(
        nc, in_maps, core_ids=list(range(N_CORES)), trace=TRACE
    )
    global LAST_RESULT
    LAST_RESULT = res
    return host_combine(res.results, cfg)
